# revision 38
# baseline (speedup 1.0000x reference)
"""GQA attention prefill (Qwen3-style) on 8 TRN2 NeuronCores.

Sharding: core c -> batch b = c // 4, kv-head pair j = c % 4
  (kv heads {2j, 2j+1}, q heads {4j..4j+3}).

Per core (engine-balanced schedule):
- Phase 1 (PE-bound ~3.4us/m): fused QKV matmuls (fp16, fp32 PSUM);
  RMSNorm stats on gpsimd (scalar_tensor_tensor accum), rstd on ACT
  (Ln/Exp), normalize+rope on DVE in fp16 (2x mode), rope-add on DVE,
  V-copy on gpsimd, Q/K transposes via XBAR dma_start_transpose.
- Phase 2 (ACT-bound): per head, per t-tile: QK matmul -> exp on ACT ->
  PV accumulation (trailing pend queue).  Head 0's scores are
  chunk-split (s 0:512 / 512:1024) and its first-chunk QK+exp are
  interleaved into phase-1's drain steps so the ACT exp stretch starts
  ~15us earlier.  Softmax denominator via fp16 pairwise adds on DVE
  plus a gpsimd partition_all_reduce (fused root+broadcast); ctx
  normalization on DVE.
- Phase 3: o-projection split into head-pair halves.  The h0+h1 half
  runs hidden under h2/h3's exp stretch; only the h2+h3 half trails the
  last exp.  Both halves are DMA'd out separately (fp16) and summed on
  the host together with the cross-core all-reduce.
"""

import numpy as np

B, S, HID = 2, 1024, 1024
NH, NKV, HD = 16, 8, 128
G = NH // NKV
CACHE_LEN, MAX_CACHE = 3072, 4096
T = CACHE_LEN + S                  # 4096
N_TT = T // 128                    # 32 t-tiles
N_CT = CACHE_LEN // 128            # 24 cached t-tiles
THETA = 1000000.0
EPS = 1e-6

_STATE = {}


def _build():
    import concourse.tile as tile
    from concourse import bacc, bass_isa, mybir

    f32 = mybir.dt.float32
    f16 = mybir.dt.float16
    AF = mybir.ActivationFunctionType
    OP = mybir.AluOpType

    nc = bacc.Bacc("TRN2", target_bir_lowering=False, debug=False, num_devices=8)

    xt_d = nc.dram_tensor("xt", [128, 8, 1024], f16, kind="ExternalInput").ap()
    wq_d = nc.dram_tensor("wq", [128, 8, 1024], f16, kind="ExternalInput").ap()
    kc_d = nc.dram_tensor("kc", [128, N_CT, 2, 128], f16, kind="ExternalInput").ap()
    vc_d = nc.dram_tensor("vc", [128, N_CT, 2, 128], f16, kind="ExternalInput").ap()
    cq_d = nc.dram_tensor("cq", [128, 8, 128], f16, kind="ExternalInput").ap()
    sq_d = nc.dram_tensor("sq", [128, 8, 128], f16, kind="ExternalInput").ap()
    ck_d = nc.dram_tensor("ck", [128, 8, 128], f16, kind="ExternalInput").ap()
    sk_d = nc.dram_tensor("sk", [128, 8, 128], f16, kind="ExternalInput").ap()
    wo_d = nc.dram_tensor("wo", [128, 4, 1024], f16, kind="ExternalInput").ap()
    tri_d = nc.dram_tensor("tri", [128, 128], f16, kind="ExternalInput").ap()
    o01_d = nc.dram_tensor("o01", [S, HID], f16, kind="ExternalOutput").ap()
    o23_d = nc.dram_tensor("o23", [S, HID], f16, kind="ExternalOutput").ap()

    with tile.TileContext(nc) as tc:
        with tc.tile_pool(name="persist", bufs=1) as persist, \
             tc.tile_pool(name="pp", bufs=24) as ppool, \
             tc.tile_pool(name="ltree", bufs=7) as ltree, \
             tc.tile_pool(name="nrm", bufs=2) as nrm, \
             tc.tile_pool(name="osb", bufs=4) as osb:
            # PSUM pools are sequenced manually (out of LIFO order):
            # phase 1: ps1(6 banks) + schalf(2); head-0 tail: schalf(2) +
            # pmix(2); heads 1-3: scf(4) + cps(2) + pmix(2).
            pmix = None
            # kT/vT: [d or tp, t-tile, kv, 128]; qT: [d, m, h, 128]
            kT = persist.tile([128, N_TT, 2, 128], f16, tag="kT")
            vT = persist.tile([128, N_TT, 2, 128], f16, tag="vT")
            qT = persist.tile([128, 8, 4, 128], f16, tag="qT")
            ctx = persist.tile([128, 4, S], f16, tag="ctx")
            wo_sb = persist.tile([128, 4, 1024], f16, tag="wo")
            tri_sb = persist.tile([128, 128], f16, tag="tri")
            # persistent causal P tiles: zero region [0:s_lo] set once
            Pc = [persist.tile([128, S], f16, tag=f"Pc{i}", name=f"Pc{i}")
                  for i in range(8)]

            eps_t = persist.tile([128, 1], f32, tag="eps")
            zero_t = persist.tile([128, 1], f32, tag="zero")
            nc.vector.memset(eps_t[:], EPS)
            nc.vector.memset(zero_t[:], 0.0)
            for ci in range(1, 8):
                nc.gpsimd.memset(Pc[ci][:, 0:128 * ci], 0.0)
            nc.scalar.add_instruction(mybir.InstLoadActFuncSet(
                name=nc.get_next_instruction_name(), ins=[], outs=[],
                act_func_set_id=6))

            # ---------------- job queues -------------------------------
            fill = []          # deferred emission jobs (FIFO)

            def pump(n):
                for _ in range(n):
                    if fill:
                        fill.pop(0)()

            # ---------------- l-tree (fp16 pairwise adds on DVE) -------
            levels = [None] * 6

            def tree_insert(P_ap, h, i):
                cur, k = P_ap, 0
                while levels[k] is not None:
                    nxt = ltree.tile([128, S], f16, tag="lv",
                                     name=f"lv{k + 1}_{h}_{i}")
                    nc.vector.tensor_add(nxt[:], levels[k][:], cur[:])
                    levels[k] = None
                    cur, k = nxt, k + 1
                levels[k] = cur

            def tree_root():
                root = None
                for k in range(6):
                    if levels[k] is None:
                        continue
                    if root is None:
                        root = levels[k]
                    else:
                        nxt = ltree.tile([128, S], f16, tag="lv",
                                         name=f"lvroot_{k}_{id(levels[k])}")
                        nc.vector.tensor_add(nxt[:], levels[k][:], root[:])
                        root = nxt
                    levels[k] = None
                return root

            schalf = tc.alloc_tile_pool(name="schalf", bufs=2, space="PSUM",
                                        side="right")
            with tc.tile_pool(name="ph1", bufs=1) as ph1, \
                 tc.tile_pool(name="tmp", bufs=3) as tmp, \
                 tc.tile_pool(name="stat", bufs=4) as statp:
                xt_sb = ph1.tile([128, 8, 1024], f16, tag="xt")
                wq_sb = ph1.tile([128, 8, 1024], f16, tag="wqkv")
                cq_sb = ph1.tile([128, 8, 128], f16, tag="cq")
                sq_sb = ph1.tile([128, 8, 128], f16, tag="sq")
                ck_sb = ph1.tile([128, 8, 128], f16, tag="ck")
                sk_sb = ph1.tile([128, 8, 128], f16, tag="sk")

                # input DMAs: xt/kc/tri on the SP queue, wq/tables/vc/wo
                # on the ACT queue (two parallel HWDGE queues).
                for kt in range(8):
                    nc.sync.dma_start(out=xt_sb[:, kt, :], in_=xt_d[:, kt, :])
                    nc.scalar.dma_start(out=wq_sb[:, kt, :], in_=wq_d[:, kt, :])
                nc.scalar.dma_start(out=cq_sb[:], in_=cq_d[:])
                nc.scalar.dma_start(out=sq_sb[:], in_=sq_d[:])
                nc.scalar.dma_start(out=ck_sb[:], in_=ck_d[:])
                nc.scalar.dma_start(out=sk_sb[:], in_=sk_d[:])
                for tch in range(4):
                    nc.sync.dma_start(
                        out=kT[:, 6 * tch:6 * (tch + 1), :, :],
                        in_=kc_d[:, 6 * tch:6 * (tch + 1), :, :])
                nc.sync.dma_start(out=tri_sb[:], in_=tri_d[:])
                nc.scalar.dma_start(out=vT[:, 0:N_CT, :, :], in_=vc_d[:])
                nc.scalar.dma_start(out=wo_sb[:], in_=wo_d[:])

                # ---------------- phase 2 emitters (head 0 early) ------
                def qk_mm(h, i, c, sc_ap, c_lo, c_hi):
                    kv = h // 2
                    m_lo = c_lo // 128
                    nc.tensor.matmul(
                        sc_ap,
                        lhsT=kT[:, i, kv, :],
                        rhs=qT[:, m_lo:c_hi // 128, h, :],
                        start=True, stop=True,
                    )

                def p_tile(h, i):
                    if i >= N_CT:
                        return Pc[i - N_CT]
                    return ppool.tile([128, S], f16, tag="P",
                                      name=f"P_{h}_{i}")

                h0_pend = []      # (i, c, c_lo, c_hi, P_t) PV jobs for head 0
                h0_P = {}

                def h0_qk_exp(i, c):
                    # chunk-split QK+exp for head 0 tile i, chunk c.  The
                    # QK matmul is high-priority so the PE backfills it
                    # between DMA-gated QKV matmuls; the exp keeps its
                    # natural (late) priority -- ACT is idle anyway.
                    s_lo = max(0, 128 * (i - N_CT))
                    c_lo, c_hi = max(s_lo, 512 * c), 512 * (c + 1)
                    if c_lo >= c_hi:
                        return
                    sc = schalf.tile([128, 512], f32, tag="sch",
                                     name=f"sch{i}_{c}")
                    with tc.high_priority(offset=1000000):
                        qk_mm(0, i, c, sc[:, 0:c_hi - c_lo], c_lo, c_hi)
                    if i not in h0_P:
                        h0_P[i] = p_tile(0, i)
                    P_t = h0_P[i]
                    nc.scalar.activation(
                        out=P_t[:, c_lo:c_hi], in_=sc[:, 0:c_hi - c_lo],
                        func=AF.Exp,
                    )
                    if i >= N_CT and s_lo + 128 <= c_hi and s_lo >= 512 * c:
                        nc.vector.tensor_mul(
                            P_t[:, s_lo:s_lo + 128],
                            P_t[:, s_lo:s_lo + 128], tri_sb[:])
                    h0_pend.append((i, c, c_lo, c_hi, P_t))

                def h0_pv(job):
                    i, c, c_lo, c_hi, P_t = job
                    last = {0: 27, 1: 31}[c]
                    nc.tensor.matmul(
                        ctx0_ps[:, c_lo:c_hi],
                        lhsT=vT[:, i, 0, :],
                        rhs=P_t[:, c_lo:c_hi],
                        start=(i == 0), stop=(i == last),
                    )

                # ---------------- Phase 1 pipeline over m ---------------
                stq = {}

                def stage_a(m):   # PE QKV matmuls
                    ps = ps1.tile([128, 1024], f32, tag="qkvps",
                                  name=f"qkvps{m}")
                    for c in range(2):
                        for kt in range(8):
                            nc.tensor.matmul(
                                ps[:, 512 * c:512 * c + 512],
                                lhsT=xt_sb[:, kt, 128 * m:128 * m + 128],
                                rhs=wq_sb[:, kt, 512 * c:512 * c + 512],
                                start=(kt == 0),
                                stop=(kt == 7),
                            )
                    stq[m] = ps

                def stage_b(m):   # Pool stats (from SBUF qn); ACT rstd
                    qn = stq[(m, "qn")]
                    sqs = tmp.tile([128, 128], f32, tag="sqs",
                                   name=f"sqs{m}", bufs=2)
                    ms = statp.tile([128, 6], f32, tag="ms", name=f"ms{m}")
                    rstd = statp.tile([128, 6], f32, tag="rstd",
                                      name=f"rstd{m}")
                    for hi in range(6):
                        nc.gpsimd.scalar_tensor_tensor(
                            out=sqs[:], in0=qn[:, 128 * hi:128 * hi + 128],
                            scalar=1.0 / HD,
                            in1=qn[:, 128 * hi:128 * hi + 128],
                            op0=OP.mult, op1=OP.mult,
                            accum_out=ms[:, hi:hi + 1],
                        )
                    # rstd = (ms + eps)^-0.5 = exp(-0.5 * ln(ms + eps))
                    nc.scalar.activation(
                        out=rstd[:], in_=ms[:], func=AF.Ln, bias=eps_t[:],
                    )
                    nc.scalar.activation(
                        out=rstd[:], in_=rstd[:], func=AF.Exp,
                        bias=zero_t[:], scale=-0.5,
                    )
                    stq[(m, "rstd")] = rstd

                def stage_e(m):   # DVE: copy PSUM out fast, rope in fp16 2x
                    ps = stq.pop(m)
                    qn = tmp.tile([128, 768], f16, tag="qn", name=f"qn{m}",
                                  bufs=2)
                    nc.vector.tensor_copy(out=qn[:], in_=ps[:, 0:768])
                    nc.vector.tensor_copy(
                        out=vT[:, N_CT + m, :, :],
                        in_=ps[:, 768:1024].rearrange("p (a b) -> p a b", a=2),
                    )
                    stq[(m, "qn")] = qn
                    ps4 = qn[:, 0:512].rearrange("p (h d) -> p h d", h=4)
                    ps2 = qn[:, 512:768].rearrange("p (h d) -> p h d", h=2)
                    t1 = tmp.tile([128, 768], f16, tag="t1", name=f"t1_{m}",
                                  bufs=2)
                    t2 = tmp.tile([128, 768], f16, tag="t2", name=f"t2_{m}",
                                  bufs=2)
                    t1q = t1[:, 0:512].rearrange("p (h d) -> p h d", h=4)
                    t1k = t1[:, 512:768].rearrange("p (h d) -> p h d", h=2)
                    t2q = t2[:, 0:512].rearrange("p (h d) -> p h d", h=4)
                    t2k = t2[:, 512:768].rearrange("p (h d) -> p h d", h=2)
                    cqb = cq_sb[:, m, :].unsqueeze(1).broadcast_to((128, 4, 128))
                    ckb = ck_sb[:, m, :].unsqueeze(1).broadcast_to((128, 2, 128))
                    sqb = sq_sb[:, m, :].unsqueeze(1).broadcast_to((128, 4, 128))
                    skb = sk_sb[:, m, :].unsqueeze(1).broadcast_to((128, 2, 128))
                    nc.vector.tensor_mul(t1q, ps4, cqb)
                    nc.vector.tensor_mul(t1k, ps2, ckb)
                    nc.vector.tensor_mul(
                        t2q[:, :, 0:64], ps4[:, :, 64:128], sqb[:, :, 0:64])
                    nc.vector.tensor_mul(
                        t2q[:, :, 64:128], ps4[:, :, 0:64], sqb[:, :, 64:128])
                    nc.vector.tensor_mul(
                        t2k[:, :, 0:64], ps2[:, :, 64:128], skb[:, :, 0:64])
                    nc.vector.tensor_mul(
                        t2k[:, :, 64:128], ps2[:, :, 0:64], skb[:, :, 64:128])
                    stq[(m, "t12")] = (t1, t2)

                def stage_f(m):   # Pool: rope combine (SBUF fp16)
                    t1, t2 = stq.pop((m, "t12"))
                    nc.gpsimd.tensor_add(t1[:], t1[:], t2[:])
                    stq[(m, "t1")] = t1

                def stage_r(m):   # DVE: per-head rstd scale (fp16 4x)
                    stq.pop((m, "qn"), None)
                    t1 = stq.pop((m, "t1"))
                    rstd = stq.pop((m, "rstd"))
                    tf = tmp.tile([128, 768], f16, tag="tf", name=f"tf{m}",
                                  bufs=2)
                    for hi in range(6):
                        sl = slice(128 * hi, 128 * hi + 128)
                        nc.vector.tensor_scalar_mul(
                            out=tf[:, sl], in0=t1[:, sl],
                            scalar1=rstd[:, hi:hi + 1],
                        )
                    stq[(m, "tf")] = tf

                def stage_g(m):   # XBAR transposes into qT / kT
                    tf = stq.pop((m, "tf"))
                    nc.sync.dma_start_transpose(
                        out=qT[:, m, :, :], in_=tf[:, 0:512])
                    nc.sync.dma_start_transpose(
                        out=kT[:, N_CT + m, :, :], in_=tf[:, 512:768])

                ps1 = tc.alloc_tile_pool(name="ps1", bufs=3, space="PSUM")
                for step in range(13):
                    if step < 8:
                        stage_a(step)
                    if 1 <= step <= 8:
                        stage_e(step - 1)
                    if 1 <= step <= 8:
                        stage_b(step - 1)
                    if 2 <= step <= 9:
                        stage_f(step - 2)
                    if 3 <= step <= 10:
                        stage_r(step - 3)
                    if 4 <= step <= 11:
                        stage_g(step - 4)
                # head 0 chunk 0: QKs are high-priority (backfill the
                # PE during the DMA-gated QKV window), exps pace ACT.
                for i in range(N_CT):
                    h0_qk_exp(i, 0)
                ps1.release()

                # ---------------- head 0: chunk 1 + causal --------------
                # ps1's banks are free now; open the mixed-use PSUM pool
                # (head-0 ctx accumulator, later the o-proj accumulators)
                pmix = tc.alloc_tile_pool(name="pmix", bufs=1, space="PSUM")
                ctx0_ps = pmix.tile([128, S], f32, tag="pmix",
                                    name="ctx_ps0")
                for i in range(N_CT):
                    h0_qk_exp(i, 1)
                    tree_insert(h0_P[i][:], 0, i)
                    while len(h0_pend) > 12:
                        h0_pv(h0_pend.pop(0))
                for i in range(N_CT, N_TT):
                    h0_qk_exp(i, 0)
                    h0_qk_exp(i, 1)
                    tree_insert(h0_P[i][:], 0, i)
                    while len(h0_pend) > 12:
                        h0_pv(h0_pend.pop(0))
                # defer the rest of head-0 PV + tail into head 1's loop
                for job in h0_pend:
                    fill.append(lambda job=job: h0_pv(job))
                h0_pend = []
                root0 = tree_root()

                def h_tail(h, root, ctx_ps):
                    box = {}

                    def t_par(half):
                        def run():
                            if half == 0:
                                box["rsum"] = nrm.tile(
                                    [128, S], f32, tag="rsum",
                                    name=f"rsum{h}", bufs=1)
                                box["rl"] = nrm.tile(
                                    [128, S], f32, tag="rl", name=f"rl{h}")
                            sl = slice(512 * half, 512 * half + 512)
                            nc.gpsimd.partition_all_reduce(
                                out_ap=box["rsum"][:, sl], in_ap=root[:, sl],
                                channels=128,
                                reduce_op=bass_isa.ReduceOp.add)
                        return run

                    def t_recip(half):
                        def run():
                            sl = slice(512 * half, 512 * half + 512)
                            nc.vector.reciprocal_approx_fast(
                                out=box["rl"][:, sl], in_=box["rsum"][:, sl])
                        return run

                    def t_mul(c4):
                        def run():
                            sl = slice(256 * c4, 256 * c4 + 256)
                            nc.vector.tensor_mul(
                                ctx[:, h, sl], ctx_ps[:, sl],
                                box["rl"][:, sl])
                        return run
                    return [t_par(0), t_recip(0), t_mul(0), t_mul(1),
                            t_par(1), t_recip(1), t_mul(2), t_mul(3)]

                for job in h_tail(0, root0, ctx0_ps):
                    fill.append(job)

            # ---------------- heads 1..3 + o-projection -----------------
            schalf.release()
            with tc.tile_pool(name="scf", bufs=2, space="PSUM") as scf, \
                 tc.tile_pool(name="cps", bufs=1, space="PSUM") as cps:

                def oproj_jobs(hpair, out_d, pools):
                    # fine-grained: one matmul per job so pump() never
                    # starves ACT; op PSUM tiles ping-pong across `pools`.
                    jobs = []
                    box = {}
                    for m in range(8):
                        def alloc(m=m):
                            pool = pools[m % len(pools)]
                            box[m] = pool.tile([128, 1024], f32,
                                               tag="pmix" if pool is pmix
                                               else "sc",
                                               name=f"op{hpair}_{m}")
                        for c2 in range(2):
                            for hh in range(2):
                                def mm(m=m, c2=c2, hh=hh):
                                    if c2 == 0 and hh == 0:
                                        alloc(m)
                                    op = box[m]
                                    h2 = 2 * hpair + hh
                                    nc.tensor.matmul(
                                        op[:, 512 * c2:512 * c2 + 512],
                                        lhsT=ctx[:, h2, 128 * m:128 * m + 128],
                                        rhs=wo_sb[:, h2,
                                                  512 * c2:512 * c2 + 512],
                                        start=(hh == 0), stop=(hh == 1),
                                    )
                                jobs.append(mm)

                        def cpy(m=m, hpair=hpair):
                            op = box.pop(m)
                            ot = osb.tile([128, 1024], f16, tag="ot",
                                          name=f"ot{hpair}_{m}")
                            if hpair == 1:
                                # ACT is idle after the last exp
                                nc.vector.tensor_copy(out=ot[:, 0:448],
                                                      in_=op[:, 0:448])
                                nc.scalar.copy(out=ot[:, 448:1024],
                                               in_=op[:, 448:1024])
                            else:
                                nc.vector.tensor_copy(out=ot[:],
                                                      in_=op[:])
                            dma_eng = nc.sync if m % 2 == 0 else nc.scalar
                            dma_eng.dma_start(
                                out=out_d[128 * m:128 * m + 128, :], in_=ot[:])
                        jobs.append(cpy)
                    return jobs

                for h in range(1, 4):
                    kv = h // 2
                    ctx_ps = cps.tile([128, S], f32, tag="ctxps",
                                      name=f"ctxps{h}")
                    pend = []

                    def pv(i, s_lo, P_t, ctx_ps=ctx_ps, kv=kv):
                        for c in range(2):
                            c_lo, c_hi = max(s_lo, 512 * c), 512 * (c + 1)
                            if c_lo >= c_hi:
                                continue
                            last_i = {0: 27, 1: 31}[c]
                            nc.tensor.matmul(
                                ctx_ps[:, c_lo:c_hi],
                                lhsT=vT[:, i, kv, :],
                                rhs=P_t[:, c_lo:c_hi],
                                start=(i == 0), stop=(i == last_i),
                            )

                    lin = {}
                    for i in range(N_TT):
                        s_lo = max(0, 128 * (i - N_CT))
                        if len(pend) > (8 if i < 12 else 2):
                            pv(*pend.pop(0))
                        if i >= 1:
                            pump(2 if h == 1 else 1)
                        sc = scf.tile([128, S], f32, tag="sc",
                                      name=f"sc{h}_{i}")
                        for c in range(2):
                            c_lo, c_hi = max(s_lo, 512 * c), 512 * (c + 1)
                            if c_lo >= c_hi:
                                continue
                            qk_mm(h, i, c, sc[:, c_lo:c_hi], c_lo, c_hi)
                        P_t = p_tile(h, i)
                        nc.scalar.activation(
                            out=P_t[:, s_lo:S], in_=sc[:, s_lo:S],
                            func=AF.Exp,
                        )
                        if i >= N_CT:
                            nc.vector.tensor_mul(
                                P_t[:, s_lo:s_lo + 128],
                                P_t[:, s_lo:s_lo + 128], tri_sb[:])
                        if h == 3 and i >= 16:
                            # linear accumulation for the last head: keeps
                            # the post-last-exp merge chain to one add
                            if "acc" not in lin:
                                lin["acc"] = tree_root()
                            nxt = ltree.tile([128, S], f16, tag="lv",
                                             name=f"lvacc_{i}")
                            nc.vector.tensor_add(nxt[:], lin["acc"][:],
                                                 P_t[:])
                            lin["acc"] = nxt
                        else:
                            tree_insert(P_t[:], h, i)
                        pend.append((i, s_lo, P_t))
                    for job in pend:
                        fill.append(lambda job=job: pv(*job))
                    root = lin["acc"] if h == 3 else tree_root()
                    for job in h_tail(h, root, ctx_ps):
                        fill.append(job)
                    if h == 1:
                        fill.extend(oproj_jobs(0, o01_d, [pmix]))
                pump(len(fill))
                for job in oproj_jobs(1, o23_d, [pmix, scf, scf]):
                    job()
            pmix.release()

    nc.compile()
    return nc


def _get_nc():
    if "nc" not in _STATE:
        _STATE["nc"] = _build()
    return _STATE["nc"]


def _host_tables(q_norm_w, k_norm_w, cache_len):
    pos = np.arange(cache_len, cache_len + S, dtype=np.float32)
    inv_freq = (1.0 / (THETA ** (np.arange(0, HD, 2, dtype=np.float32) / HD))) \
        .astype(np.float32)
    freqs = pos[:, None] * inv_freq[None, :]          # [S, 64]
    emb = np.concatenate([freqs, freqs], axis=-1)     # [S, HD]
    cos = np.cos(emb).astype(np.float32)
    sin = np.sin(emb).astype(np.float32)

    qs = np.float32(HD ** -0.5)
    cq = cos * q_norm_w[None, :] * qs
    ck = cos * k_norm_w[None, :]
    # rotate_half coefficient tables: out[d<64] += x[d+64] * (-sin[d] * w[d+64])
    #                                 out[d>=64] += x[d-64] * (sin[d] * w[d-64])
    sq = np.empty_like(sin)
    sq[:, :64] = -sin[:, :64] * q_norm_w[None, 64:]
    sq[:, 64:] = sin[:, 64:] * q_norm_w[None, :64]
    sq = sq * qs
    sk = np.empty_like(sin)
    sk[:, :64] = -sin[:, :64] * k_norm_w[None, 64:]
    sk[:, 64:] = sin[:, 64:] * k_norm_w[None, :64]

    def tile8(a):  # [S, 128] -> [128, 8, 128]
        return np.ascontiguousarray(
            a.reshape(8, 128, 128).transpose(1, 0, 2)
        ).astype(np.float16)

    return tile8(cq), tile8(sq), tile8(ck), tile8(sk)


def kernel(hidden_states, qkv_weight, q_norm_w, k_norm_w, o_weight,
           k_cache, v_cache, cache_len):
    from concourse.bass_utils import run_bass_kernel_spmd

    assert int(cache_len) == CACHE_LEN, "kernel compiled for cache_len=3072"
    hs = np.asarray(hidden_states, dtype=np.float32)
    wqkv = np.asarray(qkv_weight, dtype=np.float32)
    qnw = np.asarray(q_norm_w, dtype=np.float32)
    knw = np.asarray(k_norm_w, dtype=np.float32)
    wo = np.asarray(o_weight, dtype=np.float32)
    kc = np.asarray(k_cache, dtype=np.float32)
    vc = np.asarray(v_cache, dtype=np.float32)

    cq, sq, ck, sk = _host_tables(qnw, knw, int(cache_len))
    tri = np.triu(np.ones((128, 128), np.float32)).astype(np.float16)

    in_maps = []
    for c in range(8):
        b, j = c // 4, c % 4
        xt = np.ascontiguousarray(
            hs[b].T.reshape(8, 128, S).transpose(1, 0, 2)).astype(np.float16)
        wrows = np.concatenate([
            wqkv[512 * j:512 * j + 512],
            wqkv[2048 + 256 * j:2048 + 256 * j + 256],
            wqkv[3072 + 256 * j:3072 + 256 * j + 256],
        ], axis=0)                                     # [1024, HID]
        wq = np.ascontiguousarray(
            wrows.T.reshape(8, 128, 1024).transpose(1, 0, 2)).astype(np.float16)
        # kT cached: [d, i, kv, s_in]
        kcc = np.ascontiguousarray(
            kc[b, :CACHE_LEN, 2 * j:2 * j + 2, :]
            .reshape(N_CT, 128, 2, 128).transpose(3, 0, 2, 1)
        ).astype(np.float16)
        # vT cached: [tp, i, kv, d]
        vcc = np.ascontiguousarray(
            vc[b, :CACHE_LEN, 2 * j:2 * j + 2, :]
            .reshape(N_CT, 128, 2, 128).transpose(1, 0, 2, 3)
        ).astype(np.float16)
        wot = np.ascontiguousarray(
            wo[:, 512 * j:512 * j + 512].T.reshape(4, 128, 1024)
            .transpose(1, 0, 2)).astype(np.float16)
        in_maps.append({
            "xt": xt, "wq": wq, "kc": kcc, "vc": vcc,
            "cq": cq, "sq": sq, "ck": ck, "sk": sk,
            "wo": wot, "tri": tri,
        })

    nc = _get_nc()
    _STATE["last_in_maps"] = in_maps
    res = run_bass_kernel_spmd(nc, in_maps, core_ids=list(range(8)))
    full = np.empty((B, S, HID), np.float32)
    for b in range(B):
        acc = np.zeros((S, HID), np.float32)
        for cc in range(4):
            r = res.results[4 * b + cc]
            acc += r["o01"].astype(np.float32)
            acc += r["o23"].astype(np.float32)
        full[b] = acc
    return full
